# revision 1
# baseline (speedup 1.0000x reference)
"""DeltaNet forward kernel for Trainium2, sharded over 8 NeuronCores.

Sharding: core c handles batch c//2 and head-pair c%2 (heads {2*(c%2), 2*(c%2)+1}).
Each core computes: causal depthwise conv+silu, q/k/v/beta/g projections for its
head pair, the delta-rule recurrence via the chunked WY representation
(chunk=128, (I+A)^-1 via Neumann-series doubling), gated RMSNorm, and a partial
output projection against its 512-column slice of Wo. The host sums the two
half-DV partials per batch (row-parallel unshard).
"""

import sys

for _p in ("/opt/trn_rl_repo", "/root/.axon_site"):
    if _p not in sys.path:
        sys.path.insert(0, _p)

import numpy as np

import concourse.bass as bass
import concourse.tile as tile
from concourse import bacc, mybir
from concourse.bass_utils import run_bass_kernel_spmd
from concourse.masks import make_identity

F32 = mybir.dt.float32
F32R = mybir.dt.float32r
BF16 = mybir.dt.bfloat16

B, L, D, H = 4, 2048, 1024, 4
DK, DV = 512, 1024
HK, HV = 128, 256
CONV, EPS = 4, 1e-5
C = 128            # delta-rule chunk length
NCH = L // C       # 16 chunks
LB = 256           # L-block for projections
NLB = L // LB      # 4
KD = D // 128      # 8 contraction slices
HPC = 2            # heads per core
N_CORES = 8
QSCALE = HK ** -0.5
NEUMANN16 = True   # Tinv = sum_{k<16} M^k (else k<8)


def _mm(nc, out, lhsT, rhs, start, stop):
    """float32r matmul (full-rate 1 cycle/row when moving free dim >= 256).
    Operand tiles must be declared float32r so their producers round."""
    assert lhsT.dtype == F32R and rhs.dtype == F32R, (lhsT.dtype, rhs.dtype)
    nc.tensor.matmul(out, lhsT, rhs, start=start, stop=stop)


def build_program():
    nc = bacc.Bacc(
        "TRN2", target_bir_lowering=False, debug=False,
        enable_asserts=False, num_devices=N_CORES,
    )

    hs = nc.dram_tensor("hs", [L, D], F32, kind="ExternalInput").ap()
    cw = nc.dram_tensor("cw", [D, CONV], F32, kind="ExternalInput").ap()
    wq = nc.dram_tensor("wq", [HPC * HK, D], F32, kind="ExternalInput").ap()
    wk = nc.dram_tensor("wk", [HPC * HK, D], F32, kind="ExternalInput").ap()
    wv = nc.dram_tensor("wv", [HPC * HV, D], F32, kind="ExternalInput").ap()
    wb = nc.dram_tensor("wb", [HPC, D], F32, kind="ExternalInput").ap()
    wg = nc.dram_tensor("wg", [HPC * HV, D], F32, kind="ExternalInput").ap()
    wo = nc.dram_tensor("wo", [D, HPC * HV], F32, kind="ExternalInput").ap()
    rmsw = nc.dram_tensor("rmsw", [HV], F32, kind="ExternalInput").ap()
    y = nc.dram_tensor("y", [L, D], F32, kind="ExternalOutput").ap()

    with tile.TileContext(nc) as tc:
        _build_body(nc, tc, hs, cw, wq, wk, wv, wb, wg, wo, rmsw, y)
    nc.compile()
    return nc


def _build_body(nc, tc, hs, cw, wq, wk, wv, wb, wg, wo, rmsw, y):
    from contextlib import ExitStack

    ctx = ExitStack()
    const = ctx.enter_context(tc.tile_pool(name="const", bufs=1))
    wT = ctx.enter_context(tc.tile_pool(name="wT", bufs=1))
    wrow = ctx.enter_context(tc.tile_pool(name="wrow", bufs=3))
    dpool = ctx.enter_context(tc.tile_pool(name="dpool", bufs=2))
    ps = ctx.enter_context(tc.tile_pool(name="ps", bufs=8, space="PSUM"))
    hpool = ctx.enter_context(tc.tile_pool(name="hpool", bufs=2))
    scr = ctx.enter_context(tc.tile_pool(name="scr", bufs=3))
    xpool = ctx.enter_context(tc.tile_pool(name="xpool", bufs=2))
    hrow = ctx.enter_context(tc.tile_pool(name="hrow", bufs=2))
    qk = ctx.enter_context(tc.tile_pool(name="qk", bufs=2))
    ck = ctx.enter_context(tc.tile_pool(name="ck", bufs=3))
    ckx = ctx.enter_context(tc.tile_pool(name="ckx", bufs=6))
    otp = ctx.enter_context(tc.tile_pool(name="otp", bufs=3))
    cv = ctx.enter_context(tc.tile_pool(name="cv", bufs=3))
    sS = ctx.enter_context(tc.tile_pool(name="sS", bufs=4))
    sm = ctx.enter_context(tc.tile_pool(name="sm", bufs=6))

    # copy PSUM->SBUF on alternating engines to balance ACT/DVE load
    cp_state = [0]

    def copy_ps(dst, src):
        cp_state[0] ^= 1
        if cp_state[0]:
            nc.scalar.copy(dst, src)
        else:
            nc.vector.tensor_copy(dst, src)

    ident = const.tile([128, 128], F32)
    make_identity(nc, ident)
    epst = const.tile([128, 1], F32)
    nc.vector.memset(epst, EPS)
    identb = const.tile([128, 128], BF16)
    make_identity(nc, identb)
    # umask: 1 where free >= part (upper incl diag); numask: -1 where free > part
    umask = const.tile([128, 128], F32)
    nc.gpsimd.memset(umask, 1.0)
    nc.gpsimd.affine_select(
        out=umask, in_=umask, compare_op=mybir.AluOpType.is_ge, fill=0.0,
        base=0, channel_multiplier=-1, pattern=[[1, 128]],
    )
    numask = const.tile([128, 128], F32)
    nc.gpsimd.memset(numask, -1.0)
    nc.gpsimd.affine_select(
        out=numask, in_=numask, compare_op=mybir.AluOpType.is_gt, fill=0.0,
        base=0, channel_multiplier=-1, pattern=[[1, 128]],
    )

    def transpose_f32(in_):
        pt = ps.tile([128, 128], F32, tag="ps")
        nc.tensor.transpose(pt, in_, ident[: in_.shape[0], : in_.shape[0]])
        return pt

    # ---- constant loads ----
    cwt = const.tile([128, KD * CONV], F32)
    for d in range(KD):
        nc.sync.dma_start(
            out=cwt[:, d * CONV:(d + 1) * CONV], in_=cw[d * 128:(d + 1) * 128, :]
        )
    rmsc = const.tile([128, 2], F32)
    for s in range(2):
        nc.sync.dma_start(
            out=rmsc[:, s:s + 1],
            in_=rmsw[s * 128:(s + 1) * 128].rearrange("(p one) -> p one", one=1),
        )

    # ---- transposed weights ----
    wqT = wT.tile([128, KD, HPC * HK], F32R)   # q weights^T, pre-scaled by HK^-0.5
    wkT = wT.tile([128, KD, HPC * HK], F32R)
    wvbT = wT.tile([128, KD, HV + HPC], F32R)  # [0:256]=v head0, [256:258]=beta both
    wvT1 = wT.tile([128, KD, HV], F32R)        # v head1
    wgT = wT.tile([128, KD, HPC * HV], F32R)
    woT = wT.tile([128, 4, D], F32R)           # rms_weight folded in

    for rt in range(HPC * HK // 128):  # wq, wk: 2 row tiles each
        wr = wrow.tile([128, D], F32, tag="wrow")
        nc.sync.dma_start(out=wr, in_=wq[rt * 128:(rt + 1) * 128, :])
        wr2 = wrow.tile([128, D], F32, tag="wrow")
        nc.sync.dma_start(out=wr2, in_=wk[rt * 128:(rt + 1) * 128, :])
        for d in range(KD):
            pt = transpose_f32(wr[:, d * 128:(d + 1) * 128])
            nc.scalar.mul(wqT[:, d, rt * 128:(rt + 1) * 128], pt, QSCALE)
            pt2 = transpose_f32(wr2[:, d * 128:(d + 1) * 128])
            copy_ps(wkT[:, d, rt * 128:(rt + 1) * 128], pt2)

    for rt in range(HPC * HV // 128):  # wv: 4 row tiles
        wr = wrow.tile([128, D], F32, tag="wrow")
        nc.sync.dma_start(out=wr, in_=wv[rt * 128:(rt + 1) * 128, :])
        for d in range(KD):
            pt = transpose_f32(wr[:, d * 128:(d + 1) * 128])
            if rt < 2:
                copy_ps(wvbT[:, d, rt * 128:(rt + 1) * 128], pt)
            else:
                copy_ps(wvT1[:, d, (rt - 2) * 128:(rt - 1) * 128], pt)

    wrb = const.tile([HPC, D], F32)
    nc.sync.dma_start(out=wrb, in_=wb)
    for d in range(KD):
        pt = ps.tile([128, HPC], F32, tag="ps")
        nc.tensor.transpose(pt, wrb[:, d * 128:(d + 1) * 128], ident[:HPC, :HPC])
        copy_ps(wvbT[:, d, HV:HV + HPC], pt)

    for rt in range(HPC * HV // 128):  # wg: 4 row tiles
        wr = wrow.tile([128, D], F32, tag="wrow")
        nc.sync.dma_start(out=wr, in_=wg[rt * 128:(rt + 1) * 128, :])
        for d in range(KD):
            pt = transpose_f32(wr[:, d * 128:(d + 1) * 128])
            copy_ps(wgT[:, d, rt * 128:(rt + 1) * 128], pt)

    for rt in range(KD):  # wo: 8 row tiles of [128, 512]
        wr = wrow.tile([128, HPC * HV], F32, tag="wrow")
        nc.sync.dma_start(out=wr, in_=wo[rt * 128:(rt + 1) * 128, :])
        for s in range(4):
            pt = transpose_f32(wr[:, s * 128:(s + 1) * 128])
            nc.vector.tensor_scalar_mul(
                woT[:, s, rt * 128:(rt + 1) * 128], pt, rmsc[:, (s % 2):(s % 2) + 1]
            )

    # ---- state ----
    z256 = const.tile([128, HV], F32)
    nc.vector.memset(z256, 0.0)
    S = []
    for h in range(HPC):
        st = sS.tile([128, HV], F32R, tag="S")
        nc.scalar.copy(st, z256)
        S.append(st)

    AF = mybir.AluOpType
    ACT = mybir.ActivationFunctionType

    lbstate = {"prev_hT": None}

    def stage_lb(lb):
        # hT block: [:, d, 8:8+LB] = fresh transposed h; [:, d, 5:8] = prev tail
        hT = hpool.tile([128, KD, LB + 8], F32R, tag="hT")
        if lb > 0:
            nc.vector.tensor_copy(hT[:, :, 5:8], lbstate["prev_hT"][:, :, LB + 5:LB + 8])
        for lt in range(LB // 128):
            hr = hrow.tile([128, D], F32, tag="hrow")
            row = lb * (LB // 128) + lt
            nc.sync.dma_start(out=hr, in_=hs[row * 128:(row + 1) * 128, :])
            for d in range(KD):
                pt = transpose_f32(hr[:, d * 128:(d + 1) * 128])
                copy_ps(hT[:, d, 8 + lt * 128:8 + (lt + 1) * 128], pt)
        lbstate["prev_hT"] = hT

        # conv + silu -> xT block [128, KD, LB]
        xT = xpool.tile([128, KD, LB], F32R, tag="xT")
        for d in range(KD):
            dg = dpool.tile([128, CONV, 128], F32R, tag="dg")
            for j in range(CONV):
                nc.scalar.mul(dg[:, j, :], ident, cwt[:, d * CONV + j:d * CONV + j + 1])
            pc = ps.tile([128, LB], F32, tag="ps")
            if lb == 0:
                # first block: clip the shifted taps instead of zero-padding
                _mm(nc, pc, dg[:, 3, :], hT[:, d, 8:8 + LB], start=True, stop=False)
                for j in range(3):
                    nc.tensor.matmul(
                        pc[:, 3 - j:LB], dg[:, j, :].bitcast(F32),
                        hT[:, d, 8:8 + LB - (3 - j)].bitcast(F32),
                        start=False, stop=(j == 2))
            else:
                for j in range(CONV):
                    _mm(nc, pc, dg[:, j, :], hT[:, d, 5 + j:5 + j + LB],
                        start=(j == 0), stop=(j == CONV - 1))
            nc.scalar.activation(xT[:, d, :], pc, ACT.Silu)

        # q/k projections (T layout) for this L-block
        qT = qk.tile([128, HPC, LB], F32R, tag="qT")
        kT = qk.tile([128, HPC, LB], F32, tag="kT")
        for h in range(HPC):
            for (wt, dst) in ((wqT, qT), (wkT, kT)):
                pp = ps.tile([128, LB], F32, tag="ps")
                for ks in range(KD):
                    _mm(nc, pp, wt[:, ks, h * 128:(h + 1) * 128], xT[:, ks, :],
                        start=(ks == 0), stop=(ks == KD - 1))
                copy_ps(dst[:, h, :], pp)
        return qT, kT, xT

    def stage_a(c, qT, kT, xT):
        """Chunk-parallel work: v/g/beta projections, k-norm, A/Mqk, TinvT, -W^T."""
        ch = c % (LB // C)
        csl = slice(ch * C, (ch + 1) * C)

        pv0 = ps.tile([128, HV + HPC], F32, tag="ps")
        pv1 = ps.tile([128, HV], F32, tag="ps")
        pg = ps.tile([128, HPC * HV], F32, tag="ps")
        for ks in range(KD):
            lx = xT[:, ks, csl]
            _mm(nc, pv0, lx, wvbT[:, ks, :], start=(ks == 0), stop=(ks == KD - 1))
            _mm(nc, pv1, lx, wvT1[:, ks, :], start=(ks == 0), stop=(ks == KD - 1))
            _mm(nc, pg, lx, wgT[:, ks, :], start=(ks == 0), stop=(ks == KD - 1))
        beta = sm.tile([128, HPC], F32, tag="beta")
        nc.scalar.activation(beta, pv0[:, HV:HV + HPC], ACT.Sigmoid)
        sg = cv.tile([128, HPC * HV], F32, tag="sg")
        nc.scalar.activation(sg, pg, ACT.Silu)
        vb = cv.tile([128, HPC * HV], F32R, tag="vb")
        nc.vector.tensor_scalar_mul(vb[:, 0:HV], pv0[:, 0:HV], beta[:, 0:1])
        nc.vector.tensor_scalar_mul(vb[:, HV:2 * HV], pv1, beta[:, 1:2])

        art = {"vb": vb, "sg": sg, "qT": qT, "csl": csl, "h": []}
        for h in range(HPC):
            # --- k normalization (row space) ---
            pt = transpose_f32(kT[:, h, csl])
            kraw = ck.tile([128, 128], F32, tag="kraw")
            copy_ps(kraw, pt)
            sq = scr.tile([128, 128], F32, tag="sq")
            nsq = sm.tile([128, 1], F32, tag="nsq")
            nc.scalar.activation(sq, kraw, ACT.Square, accum_out=nsq)
            nrm = sm.tile([128, 1], F32, tag="nrm")
            nc.scalar.sqrt(nrm, nsq)
            nrm2 = sm.tile([128, 1], F32, tag="nrm2")
            nc.vector.tensor_scalar_max(nrm2, nrm, 1e-6)
            inv = sm.tile([128, 1], F32, tag="inv")
            nc.vector.reciprocal(inv, nrm2)
            knr = ckx.tile([128, 128], F32R, tag="knr")   # Kn row [C, HK]
            nc.vector.tensor_scalar_mul(knr, kraw, inv)
            kbr = ck.tile([128, 128], F32, tag="kbr")    # beta*Kn row
            nc.vector.tensor_scalar_mul(kbr, knr.bitcast(F32), beta[:, h:h + 1])
            pt = transpose_f32(knr.bitcast(F32))
            knT = ck.tile([128, 128], F32, tag="knT")
            copy_ps(knT, pt)
            pt = transpose_f32(kbr)
            kbT = ck.tile([128, 128], F32, tag="kbT")
            copy_ps(kbT, pt)

            # --- A^T = Kn Kb^T ; Mqk^T = masked Kn Q^T ---
            pA = ps.tile([128, 128], F32, tag="ps")
            nc.tensor.matmul(pA, knT, kbT, start=True, stop=True)
            pM = ps.tile([128, 128], F32, tag="ps")
            nc.tensor.matmul(pM, knT, qT[:, h, csl].bitcast(F32), start=True, stop=True)
            mqk = ckx.tile([128, 128], F32R, tag="mqk")
            nc.vector.tensor_mul(mqk, pM, umask)

            # --- TinvT = sum_k M^k, M = strict_upper(-A^T), bf16 doubling ---
            Mb = ck.tile([128, 128], BF16, tag="Mb")
            nc.vector.tensor_mul(Mb, pA, numask)
            S2 = ck.tile([128, 128], BF16, tag="S2")
            nc.vector.tensor_add(S2, Mb, identb)
            pt = ps.tile([128, 128], BF16, tag="ps")
            nc.tensor.transpose(pt, Mb, identb)
            Nb = ck.tile([128, 128], BF16, tag="Nb")
            copy_ps(Nb, pt)

            def mmb(lhsT, rhs):
                po = ps.tile([128, 128], F32, tag="ps")
                nc.tensor.matmul(po, lhsT, rhs, start=True, stop=True)
                return po

            def cast_b(po, tag):
                t = ck.tile([128, 128], BF16, tag=tag)
                copy_ps(t, po)
                return t

            P2 = cast_b(mmb(Nb, Mb), "P2")     # M @ M
            P2T = cast_b(mmb(Mb, Nb), "P2T")   # (M @ M)^T
            S4 = ck.tile([128, 128], BF16, tag="S4")
            nc.vector.tensor_add(S4, S2, mmb(P2T, S2))
            P4T = cast_b(mmb(P2, P2T), "P4T")
            if NEUMANN16:
                S8 = ck.tile([128, 128], BF16, tag="S8")
                nc.vector.tensor_add(S8, S4, mmb(P4T, S4))
                P4 = cast_b(mmb(P2T, P2), "P4")
                P8T = cast_b(mmb(P4, P4T), "P8T")
                tinvT = ckx.tile([128, 128], F32R, tag="tinvT")
                nc.vector.tensor_add(tinvT, S8, mmb(P8T, S8))
            else:
                tinvT = ckx.tile([128, 128], F32R, tag="tinvT")
                nc.vector.tensor_add(tinvT, S4, mmb(P4T, S4))

            # --- -W^T = -(Kb^T Tinv^T) ---
            pW = ps.tile([128, 128], F32, tag="ps")
            nc.tensor.matmul(pW, kbr, tinvT.bitcast(F32), start=True, stop=True)
            nWT = ckx.tile([128, 128], F32R, tag="nWT")
            nc.scalar.mul(nWT, pW, -1.0)
            art["h"].append({"knr": knr, "mqk": mqk, "tinvT": tinvT, "nWT": nWT})
        return art

    def stage_b(c, art):
        """S-dependent sequential phase + gated rmsnorm + output projection."""
        vb, sg, qT, csl = art["vb"], art["sg"], art["qT"], art["csl"]
        ofin = cv.tile([128, HPC * HV], F32, tag="ofin")
        for h in range(HPC):
            hsl = slice(h * HV, (h + 1) * HV)
            a = art["h"][h]
            # --- U = Tinv Vb - W S ---
            pU = ps.tile([128, HV], F32, tag="ps")
            _mm(nc, pU, a["nWT"], S[h], start=True, stop=False)
            _mm(nc, pU, a["tinvT"], vb[:, hsl], start=False, stop=True)
            U = cv.tile([128, HV], F32R, tag="U")
            copy_ps(U, pU)

            # --- O = Q S + Mqk U ---
            pO = ps.tile([128, HV], F32, tag="ps")
            _mm(nc, pO, qT[:, h, csl], S[h], start=True, stop=False)
            _mm(nc, pO, a["mqk"], U, start=False, stop=True)

            # --- gated rmsnorm: ofin = (O * rsqrt(mean O^2 + eps)) * silu(g)
            sq2 = scr.tile([128, HV], F32, tag="sq2")
            ms = sm.tile([128, 1], F32, tag="ms")
            nc.scalar.activation(sq2, pO, ACT.Square, accum_out=ms)
            rs1 = sm.tile([128, 1], F32, tag="rs1")
            nc.scalar.activation(rs1, ms, ACT.Sqrt, bias=epst, scale=1.0 / HV)
            rs = sm.tile([128, 1], F32, tag="rs")
            nc.vector.reciprocal(rs, rs1)
            nc.vector.scalar_tensor_tensor(
                out=ofin[:, hsl], in0=pO, scalar=rs, in1=sg[:, hsl],
                op0=AF.mult, op1=AF.mult,
            )

            # --- S += Kn^T U ---
            pD = ps.tile([128, HV], F32, tag="ps")
            _mm(nc, pD, a["knr"], U, start=True, stop=True)
            Sn = sS.tile([128, HV], F32R, tag="S")
            nc.vector.tensor_add(Sn, S[h].bitcast(F32), pD)
            S[h] = Sn

        # --- partial output projection: y[c] = ofin @ woT ---
        oT = otp.tile([128, 4, 128], F32R, tag="oT")
        for s in range(4):
            pt = transpose_f32(ofin[:, s * 128:(s + 1) * 128])
            copy_ps(oT[:, s, :], pt)
        for t2 in range(2):
            py = ps.tile([128, 512], F32, tag="ps")
            for s in range(4):
                _mm(nc, py, oT[:, s, :], woT[:, s, t2 * 512:(t2 + 1) * 512],
                    start=(s == 0), stop=(s == 3))
            yst = cv.tile([128, 512], F32, tag="yst")
            copy_ps(yst, py)
            nc.sync.dma_start(
                out=y[c * 128:(c + 1) * 128, t2 * 512:(t2 + 1) * 512], in_=yst
            )

    # software pipeline: stage A of chunk c+1 is emitted before stage B of
    # chunk c, so the PE always has independent work while the sequential
    # S-chain of the previous chunk waits on DVE/ACT results.
    CPB = LB // C
    arts = {}
    cur = None
    for c in range(NCH + 1):
        if c < NCH:
            if c % CPB == 0:
                cur = stage_lb(c // CPB)
            arts[c] = stage_a(c, *cur)
        if c >= 1:
            stage_b(c - 1, arts.pop(c - 1))

    ctx.close()


_nc_cache = None


def _get_nc():
    global _nc_cache
    if _nc_cache is None:
        _nc_cache = build_program()
    return _nc_cache


def make_in_maps(hidden_states, conv_w, Wq, Wk, Wv, Wb, Wg, Wo, rms_weight):
    arr = lambda a: np.ascontiguousarray(np.asarray(a, dtype=np.float32))
    in_maps = []
    for core in range(N_CORES):
        b, g = core // 2, core % 2
        in_maps.append({
            "hs": arr(hidden_states[b]),
            "cw": arr(conv_w),
            "wq": arr(Wq[g * HPC * HK:(g + 1) * HPC * HK]),
            "wk": arr(Wk[g * HPC * HK:(g + 1) * HPC * HK]),
            "wv": arr(Wv[g * HPC * HV:(g + 1) * HPC * HV]),
            "wb": arr(Wb[g * HPC:(g + 1) * HPC]),
            "wg": arr(Wg[g * HPC * HV:(g + 1) * HPC * HV]),
            "wo": arr(Wo[:, g * HPC * HV:(g + 1) * HPC * HV]),
            "rmsw": arr(rms_weight),
        })
    return in_maps


def unshard(results):
    y = np.empty((B, L, D), np.float32)
    for b in range(B):
        y[b] = results[2 * b]["y"] + results[2 * b + 1]["y"]
    return y


def kernel(hidden_states, conv_w, Wq, Wk, Wv, Wb, Wg, Wo, rms_weight, **_ignored):
    nc = _get_nc()
    in_maps = make_in_maps(hidden_states, conv_w, Wq, Wk, Wv, Wb, Wg, Wo, rms_weight)
    res = run_bass_kernel_spmd(nc, in_maps, core_ids=list(range(N_CORES)))
    return unshard(res.results)



# revision 5
# speedup vs baseline: 1.4600x; 1.4600x over previous
"""DeltaNet forward kernel for Trainium2, sharded over 8 NeuronCores.

Sharding: core c handles batch c//2 and head-pair c%2 (heads {2*(c%2), 2*(c%2)+1}).
Host pre-transposes all weights/activations into the layouts the device needs
(hsT padded for the causal conv, per-head-pair weight slices pre-transposed and
pre-scaled, conv weights expanded to diagonal matmul operands), so the device
does no weight transposes. Projections run in bf16 (inputs rounded on host),
the delta-rule chunk math in f32r, Neumann internals in bf16.

Single ACT table set (silu_and_others): sigmoid via 0.5+0.5*tanh(x/2), all
rsqrt via a DVE bit-trick (int shift magic + 2 Newton steps) -- no Sqrt, no
Sigmoid table loads.

Per chunk the work is split into three pipeline stages so the PE always has
S-independent work: stage_a (projections + k-norm + Tinv/W/Mqk), s_advance
(the sequential U/O/S chain), tail (gated rmsnorm + output projection), with
stage_a running 2 chunks ahead of tail.
"""

import sys

for _p in ("/opt/trn_rl_repo", "/root/.axon_site"):
    if _p not in sys.path:
        sys.path.insert(0, _p)

import numpy as np
import ml_dtypes

import concourse.bass as bass
import concourse.tile as tile
from concourse import bacc, mybir
from concourse.bass_utils import run_bass_kernel_spmd
from concourse.masks import make_identity

F32 = mybir.dt.float32
F32R = mybir.dt.float32r
BF16 = mybir.dt.bfloat16
I32 = mybir.dt.int32
NPBF = ml_dtypes.bfloat16

B, L, D, H = 4, 2048, 1024, 4
DK, DV = 512, 1024
HK, HV = 128, 256
CONV, EPS = 4, 1e-5
C = 128            # delta-rule chunk length
NCH = L // C       # 16 chunks
LB = 512           # L-block for conv / q projection
CPB = LB // C      # 4 chunks per L-block
KD = D // 128      # 8 contraction slices
HPC = 2            # heads per core
N_CORES = 8
QSCALE = HK ** -0.5
WRC = HPC * HV + HPC * HV + HPC * HK + HPC   # 1282 row-proj cols: [v|g|k|beta]
MAGIC = 0x5F3759DF


def _mm(nc, out, lhsT, rhs, start, stop):
    """float32r matmul (full-rate 1 cycle/row)."""
    assert lhsT.dtype == F32R and rhs.dtype == F32R, (lhsT.dtype, rhs.dtype)
    nc.tensor.matmul(out, lhsT, rhs, start=start, stop=stop)


def build_program():
    nc = bacc.Bacc(
        "TRN2", target_bir_lowering=False, debug=False,
        enable_asserts=False, num_devices=N_CORES,
    )

    hsT = nc.dram_tensor("hsT", [D, L + 3], BF16, kind="ExternalInput").ap()
    wq = nc.dram_tensor("wq", [D, HPC * HK], BF16, kind="ExternalInput").ap()
    wr = nc.dram_tensor("wr", [D, WRC], BF16, kind="ExternalInput").ap()
    wo = nc.dram_tensor("wo", [HPC * HV, D], BF16, kind="ExternalInput").ap()
    dgd = nc.dram_tensor("dgd", [128, KD * CONV * 128], BF16, kind="ExternalInput").ap()
    y = nc.dram_tensor("y", [L, D], F32, kind="ExternalOutput").ap()

    with tile.TileContext(nc) as tc:
        _build_body(nc, tc, hsT, wq, wr, wo, dgd, y)
    nc.compile()
    return nc


def _build_body(nc, tc, hsT, wq, wr, wo, dgd, y):
    from contextlib import ExitStack

    AF = mybir.AluOpType
    ACT = mybir.ActivationFunctionType

    ctx = ExitStack()
    const = ctx.enter_context(tc.tile_pool(name="const", bufs=1))
    ps = ctx.enter_context(tc.tile_pool(name="ps", bufs=8, space="PSUM"))
    hpool = ctx.enter_context(tc.tile_pool(name="hpool", bufs=2))
    xpool = ctx.enter_context(tc.tile_pool(name="xpool", bufs=2))
    qk = ctx.enter_context(tc.tile_pool(name="qk", bufs=2))
    sS = ctx.enter_context(tc.tile_pool(name="sS", bufs=6))
    ck = ctx.enter_context(tc.tile_pool(name="ck", bufs=3))
    ckx = ctx.enter_context(tc.tile_pool(name="ckx", bufs=5))
    cv = ctx.enter_context(tc.tile_pool(name="cv", bufs=3))
    cu = ctx.enter_context(tc.tile_pool(name="cu", bufs=4))
    otp = ctx.enter_context(tc.tile_pool(name="otp", bufs=3))
    scr = ctx.enter_context(tc.tile_pool(name="scr", bufs=3))
    sm = ctx.enter_context(tc.tile_pool(name="sm", bufs=4))

    def cp_act(dst, src):
        nc.scalar.copy(dst, src)

    def cp_dve(dst, src):
        nc.vector.tensor_copy(dst, src)

    # alternating engine for the Neumann bf16 casts
    cp_state = [0]

    def cp_alt(dst, src):
        cp_state[0] ^= 1
        (cp_act if cp_state[0] else cp_dve)(dst, src)

    # ---- constants ----
    identf = const.tile([128, 128], F32)
    make_identity(nc, identf)
    identb = const.tile([128, 128], BF16)
    make_identity(nc, identb)
    # umask: 1 where free >= part (upper incl diag); lowm: 1 where free < part
    umask = const.tile([128, 128], F32)
    nc.gpsimd.memset(umask, 1.0)
    nc.gpsimd.affine_select(
        out=umask, in_=umask, compare_op=AF.is_ge, fill=0.0,
        base=0, channel_multiplier=-1, pattern=[[1, 128]],
    )
    lowm = const.tile([128, 128], F32)   # 1 where free < part  (= 1 - umask)
    nc.vector.tensor_scalar(lowm, umask, -1.0, 1.0, AF.mult, AF.add)
    magic = const.tile([128, 2], I32)
    nc.vector.memset(magic, MAGIC)
    ones_i = const.tile([128, 1], I32)
    nc.vector.memset(ones_i, 1)

    # ---- weights (pre-transposed on host; plain DMA) ----
    wqs = const.tile([128, KD, HPC * HK], BF16)
    wrs = const.tile([128, KD, WRC], BF16)
    for ks in range(KD):
        nc.sync.dma_start(out=wqs[:, ks, :], in_=wq[ks * 128:(ks + 1) * 128, :])
        nc.sync.dma_start(out=wrs[:, ks, :], in_=wr[ks * 128:(ks + 1) * 128, :])
    wos = const.tile([128, 4, D], BF16)
    for s in range(4):
        nc.sync.dma_start(out=wos[:, s, :], in_=wo[s * 128:(s + 1) * 128, :])
    dgs = const.tile([128, KD * CONV, 128], BF16)
    nc.sync.dma_start(
        out=dgs, in_=dgd.rearrange("p (t q) -> p t q", q=128)
    )

    # ---- state ----
    S = []
    for h in range(HPC):
        st = sS.tile([128, HV], F32R, tag="S")
        nc.vector.memset(st.bitcast(F32), 0.0)
        S.append(st)

    def rsqrt2(x, n):
        """1/sqrt(x) for x [128, n] f32 SBUF via int bit-trick + 2 Newtons."""
        sh = sm.tile([128, n], I32, tag="rs_sh")
        nc.vector.tensor_scalar(
            sh, x.bitcast(I32), ones_i[:, 0:1], None, AF.logical_shift_right
        )
        y0 = sm.tile([128, n], I32, tag="rs_y0")
        nc.vector.tensor_sub(y0, magic[:, 0:n], sh)
        yv = y0.bitcast(F32)
        for it in range(2):
            t = sm.tile([128, n], F32, tag=f"rs_t{it}")
            nc.vector.tensor_mul(t, yv, yv)
            a = sm.tile([128, n], F32, tag=f"rs_a{it}")
            nc.vector.scalar_tensor_tensor(
                out=a, in0=x, scalar=-0.5, in1=t, op0=AF.mult, op1=AF.mult
            )
            yn = sm.tile([128, n], F32, tag=f"rs_y{it}")
            nc.vector.scalar_tensor_tensor(
                out=yn, in0=a, scalar=1.5, in1=yv, op0=AF.add, op1=AF.mult
            )
            yv = yn
        return yv

    def stage_lb(lb):
        """Load hsT block, causal conv + silu -> xT, q projection -> qT."""
        hT = hpool.tile([128, KD, LB + 3], BF16, tag="hT")
        for ks in range(KD):
            nc.sync.dma_start(
                out=hT[:, ks, :],
                in_=hsT[ks * 128:(ks + 1) * 128, lb * LB:lb * LB + LB + 3],
            )
        xT = xpool.tile([128, KD, LB], BF16, tag="xT")
        for d in range(KD):
            pc = ps.tile([128, LB], F32, tag="ps")
            for j in range(CONV):
                nc.tensor.matmul(
                    pc, dgs[:, d * CONV + j, :], hT[:, d, j:j + LB],
                    start=(j == 0), stop=(j == CONV - 1),
                )
            nc.scalar.activation(xT[:, d, :], pc, ACT.Silu)
        qT = qk.tile([128, HPC, LB], F32R, tag="qT")
        for h in range(HPC):
            pp = ps.tile([128, LB], F32, tag="ps")
            for ks in range(KD):
                nc.tensor.matmul(
                    pp, wqs[:, ks, h * 128:(h + 1) * 128], xT[:, ks, :],
                    start=(ks == 0), stop=(ks == KD - 1),
                )
            (cp_act if h == 0 else cp_dve)(qT[:, h, :], pp)
        return qT, xT

    def stage_a(c, qT, xT):
        """S-independent chunk work: projections, k-norm, Tinv, W, Mqk."""
        ch = c % CPB
        csl = slice(ch * C, (ch + 1) * C)

        pv = ps.tile([128, HPC * HV], F32, tag="ps")
        pg = ps.tile([128, HPC * HV], F32, tag="ps")
        pkb = ps.tile([128, HPC * HK + HPC], F32, tag="ps")
        for ks in range(KD):
            lx = xT[:, ks, csl]
            nc.tensor.matmul(pv, lx, wrs[:, ks, 0:512],
                             start=(ks == 0), stop=(ks == KD - 1))
            nc.tensor.matmul(pg, lx, wrs[:, ks, 512:1024],
                             start=(ks == 0), stop=(ks == KD - 1))
            nc.tensor.matmul(pkb, lx, wrs[:, ks, 1024:WRC],
                             start=(ks == 0), stop=(ks == KD - 1))

        # beta = sigmoid(z) = 0.5 + 0.5*tanh(z/2); nbeta = -beta
        th = sm.tile([128, HPC], F32, tag="th")
        nc.scalar.activation(th, pkb[:, 256:258], ACT.Tanh, scale=0.5)
        beta = sm.tile([128, HPC], F32, tag="beta")
        nc.vector.tensor_scalar(beta, th, 0.5, 0.5, AF.mult, AF.add)
        nbeta = sm.tile([128, HPC], F32, tag="nbeta")
        nc.vector.tensor_scalar(nbeta, th, -0.5, -0.5, AF.mult, AF.add)

        sg = cv.tile([128, HPC * HV], BF16, tag="sg")
        nc.scalar.activation(sg, pg, ACT.Silu)
        vb = cv.tile([128, HPC * HV], F32R, tag="vb")
        nc.vector.tensor_scalar_mul(vb[:, 0:HV], pv[:, 0:HV], beta[:, 0:1])
        nc.vector.tensor_scalar_mul(vb[:, HV:2 * HV], pv[:, HV:2 * HV], beta[:, 1:2])

        # k norms (both heads batched into [128, 2])
        nsq = sm.tile([128, HPC], F32, tag="nsq")
        for h in range(HPC):
            sq = scr.tile([128, 128], F32, tag="sq")
            nc.scalar.activation(
                sq, pkb[:, h * 128:(h + 1) * 128], ACT.Square,
                accum_out=nsq[:, h:h + 1],
            )
        inv = rsqrt2(nsq, HPC)
        invc = sm.tile([128, HPC], F32, tag="invc")
        nc.vector.tensor_scalar_min(invc, inv, 1e6)

        art = {"vb": vb, "sg": sg, "qT": qT, "csl": csl, "h": []}
        for h in range(HPC):
            knr = ckx.tile([128, 128], F32R, tag="knr")
            nc.vector.tensor_scalar_mul(
                knr, pkb[:, h * 128:(h + 1) * 128], invc[:, h:h + 1]
            )
            kbr = ck.tile([128, 128], F32R, tag="kbr")   # -beta * kn rows
            nc.vector.tensor_scalar_mul(kbr, knr.bitcast(F32), nbeta[:, h:h + 1])
            ptk = ps.tile([128, 128], F32, tag="ps")
            nc.tensor.transpose(ptk, knr.bitcast(F32), identf)
            knT = ck.tile([128, 128], F32R, tag="knT")
            cp_act(knT, ptk)

            # G = Kn Kn^T; Nb = strict_lower(-beta_i G) = M^T; Mb = M
            pG = ps.tile([128, 128], F32, tag="ps")
            _mm(nc, pG, knT, knT, start=True, stop=True)
            Nb = ck.tile([128, 128], BF16, tag="Nb")
            nc.vector.scalar_tensor_tensor(
                out=Nb, in0=pG, scalar=nbeta[:, h:h + 1], in1=lowm,
                op0=AF.mult, op1=AF.mult,
            )
            ptm = ps.tile([128, 128], BF16, tag="ps")
            nc.tensor.transpose(ptm, Nb, identb)
            Mb = ck.tile([128, 128], BF16, tag="Mb")
            cp_act(Mb, ptm)

            # Mqk^T = masked Kn Q^T
            pM = ps.tile([128, 128], F32, tag="ps")
            _mm(nc, pM, knT, qT[:, h, csl], start=True, stop=True)
            mqk = ckx.tile([128, 128], F32R, tag="mqk")
            nc.vector.tensor_mul(mqk, pM, umask)

            # TinvT = sum_{k<16} M^k via bf16 doubling
            S2 = ck.tile([128, 128], BF16, tag="S2")
            nc.vector.tensor_add(S2, Mb, identb)

            def mmb(lhsT, rhs):
                po = ps.tile([128, 128], F32, tag="ps")
                nc.tensor.matmul(po, lhsT, rhs, start=True, stop=True)
                return po

            def cast_b(po, tag):
                t = ck.tile([128, 128], BF16, tag=tag)
                cp_alt(t, po)
                return t

            P2 = cast_b(mmb(Nb, Mb), "P2")     # M @ M
            P2T = cast_b(mmb(Mb, Nb), "P2T")   # (M @ M)^T
            S4 = ck.tile([128, 128], BF16, tag="S4")
            nc.vector.tensor_add(S4, S2, mmb(P2T, S2))
            P4T = cast_b(mmb(P2, P2T), "P4T")
            S8 = ck.tile([128, 128], BF16, tag="S8")
            nc.vector.tensor_add(S8, S4, mmb(P4T, S4))
            P4 = cast_b(mmb(P2T, P2), "P4")
            P8T = cast_b(mmb(P4, P4T), "P8T")
            tinvT = ckx.tile([128, 128], F32R, tag="tinvT")
            nc.vector.tensor_add(tinvT, S8, mmb(P8T, S8))

            # -W^T = Kb'^T TinvT with Kb' = -beta*Kn (negative folded in kbr)
            pW = ps.tile([128, 128], F32, tag="ps")
            _mm(nc, pW, kbr, tinvT, start=True, stop=True)
            nWT = ckx.tile([128, 128], F32R, tag="nWT")
            cp_dve(nWT, pW)
            art["h"].append({"knr": knr, "mqk": mqk, "tinvT": tinvT, "nWT": nWT})
        return art

    def s_advance(c, art):
        """Sequential S-chain: U, O (matmuls only), S update."""
        vb, qT, csl = art["vb"], art["qT"], art["csl"]
        art["O"] = []
        for h in range(HPC):
            a = art["h"][h]
            hsl = slice(h * HV, (h + 1) * HV)
            pU = ps.tile([128, HV], F32, tag="ps")
            _mm(nc, pU, a["nWT"], S[h], start=True, stop=False)
            _mm(nc, pU, a["tinvT"], vb[:, hsl], start=False, stop=True)
            U = cu.tile([128, HV], F32R, tag="U")
            cp_dve(U, pU)

            pO = ps.tile([128, HV], F32, tag="ps")
            _mm(nc, pO, qT[:, h, csl], S[h], start=True, stop=False)
            _mm(nc, pO, a["mqk"], U, start=False, stop=True)
            O_s = cu.tile([128, HV], F32, tag="O")
            cp_act(O_s, pO)
            art["O"].append(O_s)

            pD = ps.tile([128, HV], F32, tag="ps")
            _mm(nc, pD, a["knr"], U, start=True, stop=True)
            Sn = sS.tile([128, HV], F32R, tag="S")
            nc.vector.tensor_add(Sn, S[h].bitcast(F32), pD)
            S[h] = Sn

    def tail(c, art):
        """Gated rmsnorm + output projection + store."""
        sg = art["sg"]
        ms = sm.tile([128, HPC], F32, tag="ms")
        for h in range(HPC):
            O_s = art["O"][h]
            sq2 = scr.tile([128, HV], F32, tag="sq2")
            nc.scalar.activation(sq2, O_s, ACT.Square, accum_out=ms[:, h:h + 1])
        msb = sm.tile([128, HPC], F32, tag="msb")
        nc.vector.tensor_scalar(msb, ms, 1.0 / HV, EPS, AF.mult, AF.add)
        rs = rsqrt2(msb, HPC)
        ofin = cv.tile([128, HPC * HV], F32, tag="ofin")
        for h in range(HPC):
            hsl = slice(h * HV, (h + 1) * HV)
            nc.vector.scalar_tensor_tensor(
                out=ofin[:, hsl], in0=art["O"][h], scalar=rs[:, h:h + 1],
                in1=sg[:, hsl], op0=AF.mult, op1=AF.mult,
            )
        oT = otp.tile([128, 4, 128], BF16, tag="oT")
        for s in range(4):
            pt = ps.tile([128, 128], F32, tag="ps")
            nc.tensor.transpose(pt, ofin[:, s * 128:(s + 1) * 128], identf)
            cp_dve(oT[:, s, :], pt)
        for t2 in range(2):
            py = ps.tile([128, 512], F32, tag="ps")
            for s in range(4):
                nc.tensor.matmul(
                    py, oT[:, s, :], wos[:, s, t2 * 512:(t2 + 1) * 512],
                    start=(s == 0), stop=(s == 3),
                )
            yst = cv.tile([128, 512], F32, tag="yst")
            cp_act(yst, py)
            nc.sync.dma_start(
                out=y[c * 128:(c + 1) * 128, t2 * 512:(t2 + 1) * 512], in_=yst
            )

    # software pipeline: stage_a(c) | s_advance(c-1) | tail(c-2)
    arts = {}
    cur = None
    for t in range(NCH + 2):
        if t < NCH:
            if t % CPB == 0:
                cur = stage_lb(t // CPB)
            arts[t] = stage_a(t, *cur)
        if 1 <= t <= NCH:
            s_advance(t - 1, arts[t - 1])
        if t >= 2:
            tail(t - 2, arts.pop(t - 2))

    ctx.close()


_nc_cache = None


def _get_nc():
    global _nc_cache
    if _nc_cache is None:
        _nc_cache = build_program()
    return _nc_cache


def make_in_maps(hidden_states, conv_w, Wq, Wk, Wv, Wb, Wg, Wo, rms_weight):
    f32 = lambda a: np.asarray(a, dtype=np.float32)
    hs, cw = f32(hidden_states), f32(conv_w)
    Wq, Wk, Wv, Wb, Wg, Wo, rmsw = (
        f32(Wq), f32(Wk), f32(Wv), f32(Wb), f32(Wg), f32(Wo), f32(rms_weight)
    )
    bf = lambda a: np.ascontiguousarray(a).astype(NPBF)

    # conv weights as diagonal matmul operands: dgd[p, (d*CONV+j)*128+q] = (p==q)*cw[d*128+p, j]
    dgd = np.zeros((128, KD * CONV * 128), np.float32)
    idx = np.arange(128)
    for d in range(KD):
        for j in range(CONV):
            dgd[idx, (d * CONV + j) * 128 + idx] = cw[d * 128:(d + 1) * 128, j]
    dgd = bf(dgd)

    rms2 = np.tile(rmsw, HPC)[:, None]  # [512, 1]
    in_maps = []
    for core in range(N_CORES):
        b, g = core // 2, core % 2
        hsT = np.zeros((D, L + 3), np.float32)
        hsT[:, 3:] = hs[b].T
        wrcat = np.concatenate(
            [
                Wv[g * HPC * HV:(g + 1) * HPC * HV].T,
                Wg[g * HPC * HV:(g + 1) * HPC * HV].T,
                Wk[g * HPC * HK:(g + 1) * HPC * HK].T,
                Wb[g * HPC:(g + 1) * HPC].T,
            ],
            axis=1,
        )  # [D, 1282]
        in_maps.append({
            "hsT": bf(hsT),
            "wq": bf(Wq[g * HPC * HK:(g + 1) * HPC * HK].T * QSCALE),
            "wr": bf(wrcat),
            "wo": bf(Wo[:, g * HPC * HV:(g + 1) * HPC * HV].T * rms2),
            "dgd": dgd,
        })
    return in_maps


def unshard(results):
    y = np.empty((B, L, D), np.float32)
    for b in range(B):
        y[b] = results[2 * b]["y"] + results[2 * b + 1]["y"]
    return y


def kernel(hidden_states, conv_w, Wq, Wk, Wv, Wb, Wg, Wo, rms_weight, **_ignored):
    nc = _get_nc()
    in_maps = make_in_maps(hidden_states, conv_w, Wq, Wk, Wv, Wb, Wg, Wo, rms_weight)
    res = run_bass_kernel_spmd(nc, in_maps, core_ids=list(range(N_CORES)))
    return unshard(res.results)


# revision 9
# speedup vs baseline: 3.0388x; 2.0815x over previous
"""DeltaNet forward kernel for Trainium2, sharded over 8 NeuronCores.

Sharding: core c handles batch c//2 and head-pair c%2 (heads {2*(c%2), 2*(c%2)+1}).
Host pre-transposes all weights/activations into the layouts the device needs
(hsT padded for the causal conv, per-head-pair weight slices pre-transposed and
pre-scaled, conv weights expanded to diagonal matmul operands), so the device
does no weight transposes. Projections run in bf16 (inputs rounded on host),
the delta-rule chunk math in f32r, Neumann internals in bf16.

Single ACT table set (silu_and_others): sigmoid via 0.5+0.5*tanh(x/2), all
rsqrt via a DVE bit-trick (int shift magic + 2 Newton steps) -- no Sqrt, no
Sigmoid table loads.

Per chunk the work is split into three pipeline stages so the PE always has
S-independent work: stage_a (projections + k-norm + Tinv/W/Mqk), s_advance
(the sequential U/O/S chain), tail (gated rmsnorm + output projection), with
stage_a running 2 chunks ahead of tail.
"""

import sys

for _p in ("/opt/trn_rl_repo", "/root/.axon_site"):
    if _p not in sys.path:
        sys.path.insert(0, _p)

import numpy as np
import ml_dtypes

import concourse.bass as bass
import concourse.tile as tile
from concourse import bacc, mybir
from concourse.bass_utils import run_bass_kernel_spmd
from concourse.masks import make_identity

F32 = mybir.dt.float32
F32R = mybir.dt.float32r
BF16 = mybir.dt.bfloat16
I32 = mybir.dt.int32
NPBF = ml_dtypes.bfloat16

B, L, D, H = 4, 2048, 1024, 4
DK, DV = 512, 1024
HK, HV = 128, 256
CONV, EPS = 4, 1e-5
C = 128            # delta-rule chunk length
NCH = L // C       # 16 chunks
LB = 512           # L-block for conv / q projection
CPB = LB // C      # 4 chunks per L-block
KD = D // 128      # 8 contraction slices
HPC = 2            # heads per core
N_CORES = 8
QSCALE = HK ** -0.5
WRC = HPC * HV + HPC * HV + HPC * HK + HPC   # 1282 row-proj cols: [v|g|k|beta]
MAGIC = 0x5F3759DF


def _mm(nc, out, lhsT, rhs, start, stop):
    """float32r matmul (full-rate 1 cycle/row)."""
    assert lhsT.dtype == F32R and rhs.dtype == F32R, (lhsT.dtype, rhs.dtype)
    nc.tensor.matmul(out, lhsT, rhs, start=start, stop=stop)


def build_program():
    nc = bacc.Bacc(
        "TRN2", target_bir_lowering=False, debug=False,
        enable_asserts=False, num_devices=N_CORES,
    )

    hsT = nc.dram_tensor("hsT", [D, L + 3], BF16, kind="ExternalInput").ap()
    wq = nc.dram_tensor("wq", [D, HPC * HK], BF16, kind="ExternalInput").ap()
    wr = nc.dram_tensor("wr", [D, WRC], BF16, kind="ExternalInput").ap()
    wo = nc.dram_tensor("wo", [HPC * HV, D], BF16, kind="ExternalInput").ap()
    dgd = nc.dram_tensor("dgd", [128, KD * CONV * 128], BF16, kind="ExternalInput").ap()
    y = nc.dram_tensor("y", [L, D], F32, kind="ExternalOutput").ap()

    with tile.TileContext(nc) as tc:
        _build_body(nc, tc, hsT, wq, wr, wo, dgd, y)
    nc.compile()
    return nc


def _build_body(nc, tc, hsT, wq, wr, wo, dgd, y):
    from contextlib import ExitStack

    AF = mybir.AluOpType
    ACT = mybir.ActivationFunctionType

    ctx = ExitStack()
    const = ctx.enter_context(tc.tile_pool(name="const", bufs=1))
    # PSUM: 8 banks split by pipeline stage so next-chunk projections never
    # wait behind the current chunk's serial Neumann/S chain.
    psP = ctx.enter_context(tc.tile_pool(name="psP", bufs=3, space="PSUM"))
    psC = ctx.enter_context(tc.tile_pool(name="psC", bufs=2, space="PSUM"))
    psS = ctx.enter_context(tc.tile_pool(name="psS", bufs=2, space="PSUM"))
    psT = ctx.enter_context(tc.tile_pool(name="psT", bufs=1, space="PSUM"))
    hpool = ctx.enter_context(tc.tile_pool(name="hpool", bufs=2))
    xpool = ctx.enter_context(tc.tile_pool(name="xpool", bufs=2))
    qk = ctx.enter_context(tc.tile_pool(name="qk", bufs=2))
    sS = ctx.enter_context(tc.tile_pool(name="sS", bufs=6))
    ck = ctx.enter_context(tc.tile_pool(name="ck", bufs=3))
    ckx = ctx.enter_context(tc.tile_pool(name="ckx", bufs=5))
    cv = ctx.enter_context(tc.tile_pool(name="cv", bufs=3))
    cu = ctx.enter_context(tc.tile_pool(name="cu", bufs=4))
    otp = ctx.enter_context(tc.tile_pool(name="otp", bufs=3))
    scr = ctx.enter_context(tc.tile_pool(name="scr", bufs=3))
    sm = ctx.enter_context(tc.tile_pool(name="sm", bufs=4))

    def cp_act(dst, src):
        nc.scalar.copy(dst, src)

    def cp_dve(dst, src):
        nc.vector.tensor_copy(dst, src)

    # alternating engine for the Neumann bf16 casts
    cp_state = [0]

    def cp_alt(dst, src):
        cp_state[0] ^= 1
        (cp_act if cp_state[0] else cp_dve)(dst, src)

    # ---- constants ----
    identf = const.tile([128, 128], F32)
    make_identity(nc, identf)
    identb = const.tile([128, 128], BF16)
    make_identity(nc, identb)
    # umask: 1 where free >= part (upper incl diag); lowm: 1 where free < part
    umask = const.tile([128, 128], F32)
    nc.gpsimd.memset(umask, 1.0)
    nc.gpsimd.affine_select(
        out=umask, in_=umask, compare_op=AF.is_ge, fill=0.0,
        base=0, channel_multiplier=-1, pattern=[[1, 128]],
    )
    lowm = const.tile([128, 128], F32)   # 1 where free < part  (= 1 - umask)
    nc.vector.tensor_scalar(lowm, umask, -1.0, 1.0, AF.mult, AF.add)
    magic = const.tile([128, 2], I32)
    nc.vector.memset(magic, MAGIC)
    ones_i = const.tile([128, 1], I32)
    nc.vector.memset(ones_i, 1)

    # ---- weights (pre-transposed on host; plain DMA) ----
    wqs = const.tile([128, KD, HPC * HK], BF16)
    wrs = const.tile([128, KD, WRC], BF16)
    for ks in range(KD):
        nc.sync.dma_start(out=wqs[:, ks, :], in_=wq[ks * 128:(ks + 1) * 128, :])
        nc.sync.dma_start(out=wrs[:, ks, :], in_=wr[ks * 128:(ks + 1) * 128, :])
    wos = const.tile([128, 4, D], BF16)
    for s in range(4):
        nc.sync.dma_start(out=wos[:, s, :], in_=wo[s * 128:(s + 1) * 128, :])
    dgs = const.tile([128, KD * CONV, 128], BF16)
    nc.sync.dma_start(
        out=dgs, in_=dgd.rearrange("p (t q) -> p t q", q=128)
    )

    # ---- state ----
    S = []
    for h in range(HPC):
        st = sS.tile([128, HV], F32R, tag="S")
        nc.vector.memset(st.bitcast(F32), 0.0)
        S.append(st)

    def rsqrt2(x, n):
        """1/sqrt(x) for x [128, n] f32 SBUF via int bit-trick + 2 Newtons."""
        sh = sm.tile([128, n], I32, tag="rs_sh")
        nc.vector.tensor_scalar(
            sh, x.bitcast(I32), ones_i[:, 0:1], None, AF.logical_shift_right
        )
        y0 = sm.tile([128, n], I32, tag="rs_y0")
        nc.vector.tensor_sub(y0, magic[:, 0:n], sh)
        yv = y0.bitcast(F32)
        for it in range(2):
            t = sm.tile([128, n], F32, tag=f"rs_t{it}")
            nc.vector.tensor_mul(t, yv, yv)
            a = sm.tile([128, n], F32, tag=f"rs_a{it}")
            nc.vector.scalar_tensor_tensor(
                out=a, in0=x, scalar=-0.5, in1=t, op0=AF.mult, op1=AF.mult
            )
            yn = sm.tile([128, n], F32, tag=f"rs_y{it}")
            nc.vector.scalar_tensor_tensor(
                out=yn, in0=a, scalar=1.5, in1=yv, op0=AF.add, op1=AF.mult
            )
            yv = yn
        return yv

    def stage_lb(lb):
        """Load hsT block, causal conv + silu -> xT, q projection -> qT."""
        hT = hpool.tile([128, KD, LB + 3], BF16, tag="hT")
        for ks in range(KD):
            nc.sync.dma_start(
                out=hT[:, ks, :],
                in_=hsT[ks * 128:(ks + 1) * 128, lb * LB:lb * LB + LB + 3],
            )
        xT = xpool.tile([128, KD, LB], BF16, tag="xT")
        for d in range(KD):
            pc = psP.tile([128, LB], F32, tag="psP")
            for j in range(CONV):
                nc.tensor.matmul(
                    pc, dgs[:, d * CONV + j, :], hT[:, d, j:j + LB],
                    start=(j == 0), stop=(j == CONV - 1),
                )
            nc.scalar.activation(xT[:, d, :], pc, ACT.Silu)
        qT = qk.tile([128, HPC, LB], F32R, tag="qT")
        for h in range(HPC):
            pp = psP.tile([128, LB], F32, tag="psP")
            for ks in range(KD):
                nc.tensor.matmul(
                    pp, wqs[:, ks, h * 128:(h + 1) * 128], xT[:, ks, :],
                    start=(ks == 0), stop=(ks == KD - 1),
                )
            (cp_act if h == 0 else cp_dve)(qT[:, h, :], pp)
        return qT, xT

    def stage_a(c, qT, xT):
        """S-independent chunk work: projections, k-norm, Tinv, W, Mqk."""
        ch = c % CPB
        csl = slice(ch * C, (ch + 1) * C)

        # k/beta projection first: its (serial) norm chain overlaps the
        # v/g projection matmuls that follow.
        pkb = psP.tile([128, HPC * HK + HPC], F32, tag="psP")
        for ks in range(KD):
            nc.tensor.matmul(pkb, xT[:, ks, csl], wrs[:, ks, 1024:WRC],
                             start=(ks == 0), stop=(ks == KD - 1))
        # beta = sigmoid(z) = 0.5 + 0.5*tanh(z/2); nbeta = -beta
        th = sm.tile([128, HPC], F32, tag="th")
        nc.scalar.activation(th, pkb[:, 256:258], ACT.Tanh, scale=0.5)
        beta = sm.tile([128, HPC], F32, tag="beta")
        nc.vector.tensor_scalar(beta, th, 0.5, 0.5, AF.mult, AF.add)
        nbeta = sm.tile([128, HPC], F32, tag="nbeta")
        nc.vector.tensor_scalar(nbeta, th, -0.5, -0.5, AF.mult, AF.add)
        # k norms (both heads batched into [128, 2])
        nsq = sm.tile([128, HPC], F32, tag="nsq")
        for h in range(HPC):
            sq = scr.tile([128, 128], F32, tag="sq")
            nc.scalar.activation(
                sq, pkb[:, h * 128:(h + 1) * 128], ACT.Square,
                accum_out=nsq[:, h:h + 1],
            )
        inv = rsqrt2(nsq, HPC)
        invc = sm.tile([128, HPC], F32, tag="invc")
        nc.vector.tensor_scalar_min(invc, inv, 1e6)

        pv = psP.tile([128, HPC * HV], F32, tag="psP")
        pg = psP.tile([128, HPC * HV], F32, tag="psP")
        for ks in range(KD):
            lx = xT[:, ks, csl]
            nc.tensor.matmul(pv, lx, wrs[:, ks, 0:512],
                             start=(ks == 0), stop=(ks == KD - 1))
            nc.tensor.matmul(pg, lx, wrs[:, ks, 512:1024],
                             start=(ks == 0), stop=(ks == KD - 1))
        sg = cv.tile([128, HPC * HV], BF16, tag="sg")
        nc.scalar.activation(sg, pg, ACT.Silu)
        vb = cv.tile([128, HPC * HV], F32R, tag="vb")
        nc.vector.tensor_scalar_mul(vb[:, 0:HV], pv[:, 0:HV], beta[:, 0:1])
        nc.vector.tensor_scalar_mul(vb[:, HV:2 * HV], pv[:, HV:2 * HV], beta[:, 1:2])

        art = {"vb": vb, "sg": sg, "qT": qT, "csl": csl, "h": []}
        for h in range(HPC):
            knr = ckx.tile([128, 128], F32R, tag="knr")
            nc.vector.tensor_scalar_mul(
                knr, pkb[:, h * 128:(h + 1) * 128], invc[:, h:h + 1]
            )
            kbr = ck.tile([128, 128], F32R, tag="kbr")   # -beta * kn rows
            nc.vector.tensor_scalar_mul(kbr, knr.bitcast(F32), nbeta[:, h:h + 1])
            ptk = psC.tile([128, 128], F32, tag="psC")
            nc.tensor.transpose(ptk, knr.bitcast(F32), identf)
            knT = ck.tile([128, 128], F32R, tag="knT")
            cp_act(knT, ptk)

            # G = Kn Kn^T; Nb = strict_lower(-beta_i G) = M^T; Mb = M
            pG = psC.tile([128, 128], F32, tag="psC")
            _mm(nc, pG, knT, knT, start=True, stop=True)
            Nb = ck.tile([128, 128], BF16, tag="Nb")
            nc.vector.scalar_tensor_tensor(
                out=Nb, in0=pG, scalar=nbeta[:, h:h + 1], in1=lowm,
                op0=AF.mult, op1=AF.mult,
            )
            ptm = psC.tile([128, 128], BF16, tag="psC")
            nc.tensor.transpose(ptm, Nb, identb)
            Mb = ck.tile([128, 128], BF16, tag="Mb")
            cp_act(Mb, ptm)

            # Mqk^T = masked Kn Q^T
            pM = psC.tile([128, 128], F32, tag="psC")
            _mm(nc, pM, knT, qT[:, h, csl], start=True, stop=True)
            mqk = ckx.tile([128, 128], F32R, tag="mqk")
            nc.vector.tensor_mul(mqk, pM, umask)

            # TinvT = sum_{k<8} M^k via bf16 doubling
            S2 = ck.tile([128, 128], BF16, tag="S2")
            nc.vector.tensor_add(S2, Mb, identb)

            def mmb(lhsT, rhs):
                po = psC.tile([128, 128], F32, tag="psC")
                nc.tensor.matmul(po, lhsT, rhs, start=True, stop=True)
                return po

            def cast_b(po, tag):
                t = ck.tile([128, 128], BF16, tag=tag)
                cp_alt(t, po)
                return t

            P2 = cast_b(mmb(Nb, Mb), "P2")     # M @ M
            P2T = cast_b(mmb(Mb, Nb), "P2T")   # (M @ M)^T
            S4 = ck.tile([128, 128], BF16, tag="S4")
            nc.vector.tensor_add(S4, S2, mmb(P2T, S2))
            P4T = cast_b(mmb(P2, P2T), "P4T")
            tinvT = ckx.tile([128, 128], F32R, tag="tinvT")
            nc.vector.tensor_add(tinvT, S4, mmb(P4T, S4))

            # -W^T = Kb'^T TinvT with Kb' = -beta*Kn (negative folded in kbr)
            pW = psC.tile([128, 128], F32, tag="psC")
            _mm(nc, pW, kbr, tinvT, start=True, stop=True)
            nWT = ckx.tile([128, 128], F32R, tag="nWT")
            cp_dve(nWT, pW)
            art["h"].append({"knr": knr, "mqk": mqk, "tinvT": tinvT, "nWT": nWT})
        return art

    def s_advance(c, art):
        """Sequential S-chain: U, O (matmuls only), S update."""
        vb, qT, csl = art["vb"], art["qT"], art["csl"]
        art["O"] = []
        for h in range(HPC):
            a = art["h"][h]
            hsl = slice(h * HV, (h + 1) * HV)
            pU = psS.tile([128, HV], F32, tag="psS")
            _mm(nc, pU, a["nWT"], S[h], start=True, stop=False)
            _mm(nc, pU, a["tinvT"], vb[:, hsl], start=False, stop=True)
            U = cu.tile([128, HV], F32R, tag="U")
            cp_dve(U, pU)

            pO = psS.tile([128, HV], F32, tag="psS")
            _mm(nc, pO, qT[:, h, csl], S[h], start=True, stop=False)
            _mm(nc, pO, a["mqk"], U, start=False, stop=True)
            O_s = cu.tile([128, HV], F32, tag="O")
            cp_act(O_s, pO)
            art["O"].append(O_s)

            pD = psS.tile([128, HV], F32, tag="psS")
            _mm(nc, pD, a["knr"], U, start=True, stop=True)
            Sn = sS.tile([128, HV], F32R, tag="S")
            nc.vector.tensor_add(Sn, S[h].bitcast(F32), pD)
            S[h] = Sn

    def tail(c, art):
        """Gated rmsnorm + output projection + store."""
        sg = art["sg"]
        ms = sm.tile([128, HPC], F32, tag="ms")
        for h in range(HPC):
            O_s = art["O"][h]
            sq2 = scr.tile([128, HV], F32, tag="sq2")
            nc.scalar.activation(sq2, O_s, ACT.Square, accum_out=ms[:, h:h + 1])
        msb = sm.tile([128, HPC], F32, tag="msb")
        nc.vector.tensor_scalar(msb, ms, 1.0 / HV, EPS, AF.mult, AF.add)
        rs = rsqrt2(msb, HPC)
        ofin = cv.tile([128, HPC * HV], F32, tag="ofin")
        for h in range(HPC):
            hsl = slice(h * HV, (h + 1) * HV)
            nc.vector.scalar_tensor_tensor(
                out=ofin[:, hsl], in0=art["O"][h], scalar=rs[:, h:h + 1],
                in1=sg[:, hsl], op0=AF.mult, op1=AF.mult,
            )
        oT = otp.tile([128, 4, 128], BF16, tag="oT")
        for s in range(4):
            pt = psT.tile([128, 128], F32, tag="psT")
            nc.tensor.transpose(pt, ofin[:, s * 128:(s + 1) * 128], identf)
            cp_dve(oT[:, s, :], pt)
        for t2 in range(2):
            py = psT.tile([128, 512], F32, tag="psT")
            for s in range(4):
                nc.tensor.matmul(
                    py, oT[:, s, :], wos[:, s, t2 * 512:(t2 + 1) * 512],
                    start=(s == 0), stop=(s == 3),
                )
            yst = cv.tile([128, 512], F32, tag="yst")
            cp_act(yst, py)
            nc.sync.dma_start(
                out=y[c * 128:(c + 1) * 128, t2 * 512:(t2 + 1) * 512], in_=yst
            )

    # software pipeline: stage_a(c) | s_advance(c-1) | tail(c-2)
    arts = {}
    cur = None
    for t in range(NCH + 2):
        if t < NCH:
            if t % CPB == 0:
                cur = stage_lb(t // CPB)
            arts[t] = stage_a(t, *cur)
        if 1 <= t <= NCH:
            s_advance(t - 1, arts[t - 1])
        if t >= 2:
            tail(t - 2, arts.pop(t - 2))

    ctx.close()


_nc_cache = None


def _get_nc():
    global _nc_cache
    if _nc_cache is None:
        _nc_cache = build_program()
    return _nc_cache


def make_in_maps(hidden_states, conv_w, Wq, Wk, Wv, Wb, Wg, Wo, rms_weight):
    f32 = lambda a: np.asarray(a, dtype=np.float32)
    hs, cw = f32(hidden_states), f32(conv_w)
    Wq, Wk, Wv, Wb, Wg, Wo, rmsw = (
        f32(Wq), f32(Wk), f32(Wv), f32(Wb), f32(Wg), f32(Wo), f32(rms_weight)
    )
    bf = lambda a: np.ascontiguousarray(a).astype(NPBF)

    # conv weights as diagonal matmul operands: dgd[p, (d*CONV+j)*128+q] = (p==q)*cw[d*128+p, j]
    dgd = np.zeros((128, KD * CONV * 128), np.float32)
    idx = np.arange(128)
    for d in range(KD):
        for j in range(CONV):
            dgd[idx, (d * CONV + j) * 128 + idx] = cw[d * 128:(d + 1) * 128, j]
    dgd = bf(dgd)

    rms2 = np.tile(rmsw, HPC)[:, None]  # [512, 1]
    in_maps = []
    for core in range(N_CORES):
        b, g = core // 2, core % 2
        hsT = np.zeros((D, L + 3), np.float32)
        hsT[:, 3:] = hs[b].T
        wrcat = np.concatenate(
            [
                Wv[g * HPC * HV:(g + 1) * HPC * HV].T,
                Wg[g * HPC * HV:(g + 1) * HPC * HV].T,
                Wk[g * HPC * HK:(g + 1) * HPC * HK].T,
                Wb[g * HPC:(g + 1) * HPC].T,
            ],
            axis=1,
        )  # [D, 1282]
        in_maps.append({
            "hsT": bf(hsT),
            "wq": bf(Wq[g * HPC * HK:(g + 1) * HPC * HK].T * QSCALE),
            "wr": bf(wrcat),
            "wo": bf(Wo[:, g * HPC * HV:(g + 1) * HPC * HV].T * rms2),
            "dgd": dgd,
        })
    return in_maps


def unshard(results):
    y = np.empty((B, L, D), np.float32)
    for b in range(B):
        y[b] = results[2 * b]["y"] + results[2 * b + 1]["y"]
    return y


def kernel(hidden_states, conv_w, Wq, Wk, Wv, Wb, Wg, Wo, rms_weight, **_ignored):
    nc = _get_nc()
    in_maps = make_in_maps(hidden_states, conv_w, Wq, Wk, Wv, Wb, Wg, Wo, rms_weight)
    res = run_bass_kernel_spmd(nc, in_maps, core_ids=list(range(N_CORES)))
    return unshard(res.results)
